# revision 73
# baseline (speedup 1.0000x reference)
"""Trainium2 Bass kernel for 5x5x5 all-ones Conv3d (box filter), stride 1, pad 2.

Input x: (4, 1, 128, 256, 256) fp32, W: (1,1,5,5,5) all-ones.
Output:  (4, 1, 128, 256, 256) fp32.

Strategy (8 NeuronCores): shard batch(4) x H-halves(2) -> 8 shards. The
all-ones conv is separable into three 5-tap box sums (W, H, D).

Pipeline (int8 input / fp16 output; fp32 where precision needs it):
  - Input quantized to int8 on host (one global scale s8 = absmax/127,
    folded into the band matrix), output fp16: 1 + 2 bytes per voxel of
    HBM traffic vs 4 + 4 fp32 -> per-core DMA ~12.9 MB ~36 us at the
    modeled 360 B/ns. All device arithmetic on exact small integers
    (scan state fp32, a/c sums < 2048 exact in fp16, PSUM fp32), so the
    only model error is the int8 quantization itself (~1.23e-2 norm).
    Set INPUT_DT="f16" for a ~5e-4-accurate variant (+12 us DMA).
  - W-axis 5-tap box in ONE DVE pass: tensor_tensor_scan with
    state = (x[t] + state) - x[t-5]  (data1 = the stream shifted by 5;
    the shift opens on zeroed lead columns in the chunk tile). The scan
    ISA op only exists on DVE, so DVE's ~36 us of scans is the floor
    the rest of the schedule is balanced around. Each chunk scans in
    two sub-scans (split at 7 rows, each with its own zero lead) so the
    c-pass and matmuls start before the whole chunk is scanned.
  - H-axis 5-tap = 3 PSUM-accumulating matmuls per 2-row group using
    pair sums: box5_h = c_{h-2} + c_h + a_{h+2},  c_h = a_h + a_{h+1}.
    Each matmul contracts D with the clipped banded all-ones matrix
    (s8-scaled) -> D-axis 5-tap fused into the same matmuls.
  - The c-pass is split column-wise: cols [0,96) on DVE (0.52 ns/elem
    in the 2x fp16 mode), cols [96,256) on Pool (gpsimd Add, ~1.98
    ns/elem) -> DVE ~45 us, Pool ~42 us, PE ~43 us, ACT ~35 us all
    land just under/around the pipeline's ~58 us span.
  - a (scan stream) and c live in persistent SBUF buffers -> chunk
    halos are plain backward reads, no copy traffic.
  - ScalarE (ACT) evicts 4-row PSUM tiles -> fp16 out tiles; in-DMA on
    the SP HWDGE ring, out-DMA (split at 8 rows) on the ACT ring.

Cost-model TimelineSim: 58.0 us/core (baseline fp32 kernel: 101.6 us).
Relative error 1.232e-2 (int8 quantization), under the 2e-2 gate.
"""

import numpy as np

import concourse.mybir as mybir
import concourse.tile as tile
from concourse import bacc
from concourse.bass_utils import run_bass_kernel_spmd

# Problem geometry (hardcoded; kernel.py must be self-contained).
B = 4
DEP = 128                  # depth (on partitions)
HGT = 256                  # height
WID = 256                  # width
KS = 5
R = 2                      # conv radius

N_CORES = 8
H_HALF = HGT // 2          # 128 output rows per core
H_IN = H_HALF + 2 * R      # 132 input rows per core
W_PAD = WID + 2 * R        # 260

# Per-chunk output rows (sum = 128). Tapered ends shrink pipeline
# fill/drain. The W-box scan is DVE-only (the scan ISA op doesn't
# exist on Pool); the c-pass is split column-wise between DVE
# (0.52 ns/elem, 2x fp16 mode) and Pool (~1.98 ns/elem gpsimd Add) so
# no engine exceeds the ~5.2 us/chunk steady pace.
CHUNKS = [4, 12, 16, 16, 16, 16, 16, 16, 8, 6, 2]
XT_MAX = 16                # max newly-loaded x rows per chunk
U_COLS = 0                 # columns [0, U): direct 5-tap matmuls, no c.
                           # Tested at 24: DVE busy drops as expected,
                           # but total worsens - PE's finish is
                           # onset+work bound and PE already trails DVE,
                           # so adding PE work extends the tail.
C_DVE_COLS = 96            # c columns [U_COLS, 96) on DVE; rest on Pool
SET_ROWS = 4               # rows per PSUM tile (2 banks; 2-row matmuls,
                           # one 4-row ACT evict)
PE_WARMUP = 0              # dummy matmuls pre-ramping the PE p-state
TAIL_FILL = True           # last two chunks' c all on DVE

INPUT_DT = "i8"            # "i8" (int8 input) | "f16"
REPEAT = 1                 # run the whole body N times (benchmarking only)
TRACE = False              # set True (from test.py) to profile
LAST_RESULT = None         # BassKernelResults of the last run (for test.py)

_NC_CACHE = {}

F16 = mybir.dt.float16


def _nonce_cols():
    key = (REPEAT, tuple(CHUNKS), C_DVE_COLS, INPUT_DT, SET_ROWS,
           PE_WARMUP, "v5")
    return 8 + hash(key) % 4093


def _build_nc():
    """Build the per-core Bass program (identical on all 8 cores)."""
    nc = bacc.Bacc("TRN2", target_bir_lowering=False, debug=False)

    in_dt = mybir.dt.int8 if INPUT_DT == "i8" else F16
    x_d = nc.dram_tensor("x", [DEP, H_IN, W_PAD], in_dt, kind="ExternalInput")
    band_d = nc.dram_tensor("band", [DEP, DEP], F16, kind="ExternalInput")
    # unused input whose shape encodes the config -> distinct HLO fingerprint
    # per kernel variant (defeats any shape-keyed executable caching)
    nc.dram_tensor("nonce", [1, _nonce_cols()], mybir.dt.float32,
                   kind="ExternalInput")
    y_d = nc.dram_tensor("y", [DEP, H_HALF, WID], F16, kind="ExternalOutput")

    ASTR_N = H_IN * W_PAD + 16   # slop: row-view slices of the last rows

    with tile.TileContext(nc) as tc:
        with (
            tc.tile_pool(name="const", bufs=1) as cpool,
            tc.tile_pool(name="xin", bufs=3) as xin_pool,
            tc.tile_pool(name="opool", bufs=4) as out_pool,
            tc.tile_pool(name="psum", bufs=4, space="PSUM") as ps_pool,
        ):
            band = cpool.tile([DEP, DEP], F16, name="band")

            # Persistent scan stream: a_h (W-box of x row hx=h+2) lives at
            # astr[hx*W_PAD + 4 + w], w in [0, WID). Persistent c buffer:
            # c_h = a_h + a_{h+1} at cbuf[:, h+2, :], h in [-2, 128).
            astr = cpool.tile([DEP, ASTR_N], F16, name="astr")
            cbuf = cpool.tile([DEP, H_HALF + 2, WID], F16, name="cbuf")

            # xin tile layout: [lead8][rows 0..7)[lead8][rows 7..). Both
            # 8-col zero leads let the two sub-scans run independently
            # (each data1 window opens on zeros). Chunk DMAs never write
            # the leads, so zeroing each pool buffer ONCE suffices.
            LEAD2 = 8 + 7 * W_PAD        # second lead offset (split at 7)
            XT_N = 16 + XT_MAX * W_PAD
            for _ in range(3):
                xt0 = xin_pool.tile([DEP, XT_N], in_dt, name="xt", tag="xt")
                nc.vector.memset(xt0[:, 0:8], 0.0)
                nc.vector.memset(xt0[:, LEAD2:LEAD2 + 8], 0.0)

            if PE_WARMUP:
                # PE p-state warmup: the cost model ramps the Tensor
                # engine (LOW -> MID -> full over ~3 us of continuous
                # work); zero x zero dummy matmuls through the fill
                # phase let the first real matmuls run at full rate.
                # Cost-neutral end-to-end (PE's finish is phase-bound),
                # so disabled by default.
                wt0 = cpool.tile([DEP, DEP], F16, name="wt0")
                rhs0 = cpool.tile([DEP, 512], F16, name="rhs0")
                nc.gpsimd.memset(wt0[:], 0.0)
                nc.gpsimd.memset(rhs0[:], 0.0)
                for _ in range(PE_WARMUP):
                    psw = ps_pool.tile([DEP, SET_ROWS, WID],
                                       mybir.dt.float32, name="ps",
                                       tag="ps")
                    nc.tensor.matmul(psw[:, 0:2, :], wt0[:], rhs0[:],
                                     start=True, stop=True)

            def arows(hx0, nr):
                """[DEP, nr, WID] view of a rows hx0..hx0+nr (x-row idx)."""
                b0 = hx0 * W_PAD + 4
                v = astr[:, b0:b0 + nr * W_PAD]
                return v.rearrange("q (r w) -> q r w", r=nr, w=W_PAD)[:, :, 0:WID]

            h0 = 0
            loaded = 0      # x rows loaded/scanned so far
            c_lo = -R       # next c row to compute
            for idx, oc in enumerate(CHUNKS * REPEAT):
                if idx % len(CHUNKS) == 0:
                    h0 = 0
                    loaded = 0
                    c_lo = -R

                # ---- load new x rows + W-box scan into astr (DVE) ----
                # Scans are independent at row boundaries: boxes never
                # span rows, and each sub-scan's data1 window opens on
                # its own zeroed lead columns, so the shifted-difference
                # recurrence starts clean (initial=0) in every sub-scan.
                hx_end = h0 + oc + 2 * R
                n_new = hx_end - loaded
                s0 = loaded
                xt = xin_pool.tile([DEP, XT_N], in_dt, name="xt", tag="xt")
                kA = min(n_new, 7)
                nc.sync.dma_start(out=xt[:, 8:8 + kA * W_PAD],
                                  in_=x_d[:, s0:s0 + kA, :])
                if kA < n_new:
                    nc.sync.dma_start(
                        out=xt[:, LEAD2 + 8:
                               LEAD2 + 8 + (n_new - kA) * W_PAD],
                        in_=x_d[:, s0 + kA:s0 + n_new, :])

                def sub_scan(r0, nr, base):
                    """Scan x rows [s0+r0, s0+r0+nr); `base` is the
                    tile offset of row r0's data (preceded by zeros)."""
                    nf = nr * W_PAD
                    p0 = (s0 + r0) * W_PAD
                    nc.vector.tensor_tensor_scan(
                        out=astr[:, p0:p0 + nf],
                        data0=xt[:, base:base + nf],
                        data1=xt[:, base - 5:base - 5 + nf],
                        initial=0.0,
                        op0=mybir.AluOpType.add,
                        op1=mybir.AluOpType.subtract,
                    )

                # all-DVE c for chunk 0 (Pool launch off the fill chain)
                # and optionally the last two chunks (Pool's slow final
                # c otherwise extends the drain cascade)
                ci = idx % len(CHUNKS)
                fill = ci == 0 or (TAIL_FILL and ci >= len(CHUNKS) - 2)

                def emit_c(upto):
                    """c rows [c_lo, upto): one DVE instruction for its
                    column share (fewer per-instruction inits; DVE never
                    gates the matmuls), <=8-row pieces for Pool's share
                    (Pool is slower, so its first piece should unblock
                    the chunk's first matmul sets early)."""
                    nonlocal c_lo
                    if c_lo >= upto:
                        return
                    nr_all = upto - c_lo
                    split = WID if fill else C_DVE_COLS
                    # cols [0, U_COLS) skip c entirely (5-tap zone)
                    nc.vector.tensor_add(
                        out=cbuf[:, c_lo + 2:c_lo + 2 + nr_all,
                                 U_COLS:split],
                        in0=arows(c_lo + 2, nr_all)[:, :, U_COLS:split],
                        in1=arows(c_lo + 3, nr_all)[:, :, U_COLS:split],
                    )
                    while c_lo < upto:
                        nr = min(8, upto - c_lo)
                        if split < WID:
                            nc.gpsimd.tensor_add(
                                out=cbuf[:, c_lo + 2:c_lo + 2 + nr,
                                         split:WID],
                                in0=arows(c_lo + 2, nr)[:, :, split:WID],
                                in1=arows(c_lo + 3, nr)[:, :, split:WID],
                            )
                        c_lo += nr

                c_hi = h0 + oc
                sub_scan(0, kA, 8)
                # c rows coverable with a rows hx < s0+kA: h+3 <= s0+kA
                emit_c(min(c_hi, s0 + kA - 3))
                if kA < n_new:
                    sub_scan(kA, n_new - kA, LEAD2 + 8)
                emit_c(c_hi)
                loaded = hx_end
                if idx == 0:
                    # band via the Pool SWDGE path: keeps the shared
                    # HWDGE queue free for chunk 0's x DMA (x is on the
                    # critical path; band isn't needed until the first
                    # matmul)
                    nc.gpsimd.dma_start(out=band[:], in_=band_d[:])

                # ---- D-sum + H-sum: 3 accumulating matmuls per 2-row
                # group; 4-row PSUM tiles evicted in one ACT copy ----
                out_t = out_pool.tile([DEP, 16, WID], F16,
                                      name="out_t", tag="out_t")
                r0 = h0
                while r0 < h0 + oc:
                    rows = min(SET_ROWS, h0 + oc - r0)
                    ps = ps_pool.tile([DEP, SET_ROWS, WID],
                                      mybir.dt.float32, name="ps", tag="ps")
                    for g0 in range(0, rows, 2):
                        gr = min(2, rows - g0)
                        r = r0 + g0
                        if U_COLS:
                            # cols [0, U_COLS): 5 direct a-taps (no c)
                            pu = ps[:, g0:g0 + gr, 0:U_COLS]
                            for j in range(KS):
                                nc.tensor.matmul(
                                    pu, band[:],
                                    arows(r + j, gr)[:, :, 0:U_COLS],
                                    start=(j == 0), stop=(j == KS - 1))
                        # cols [U_COLS, WID): c_{h-2}, c_h, a_{h+2}
                        pv = ps[:, g0:g0 + gr, U_COLS:WID]
                        nc.tensor.matmul(pv, band[:],
                                         cbuf[:, r:r + gr, U_COLS:WID],
                                         start=True, stop=False)
                        nc.tensor.matmul(pv, band[:],
                                         cbuf[:, r + 2:r + 2 + gr,
                                              U_COLS:WID],
                                         start=False, stop=False)
                        nc.tensor.matmul(pv, band[:],
                                         arows(r + 4, gr)[:, :,
                                                          U_COLS:WID],
                                         start=False, stop=True)
                    nc.scalar.copy(out=out_t[:, r0 - h0:r0 - h0 + rows, :],
                                   in_=ps[:, 0:rows, :])
                    r0 += rows
                    # out-DMA on the ACT HWDGE ring (separate FIFO from
                    # in-DMAs); split at 8 rows so the first half ships
                    # while the second half is still in the matmuls
                    if r0 - h0 == 8 and oc > 8:
                        nc.scalar.dma_start(out=y_d[:, h0:h0 + 8, :],
                                            in_=out_t[:, 0:8, :])
                done = 8 if oc > 8 else 0
                nc.scalar.dma_start(out=y_d[:, h0 + done:h0 + oc, :],
                                    in_=out_t[:, done:oc, :])
                h0 += oc

    return nc


def _get_nc():
    key = (REPEAT, tuple(CHUNKS), C_DVE_COLS, INPUT_DT, SET_ROWS,
           PE_WARMUP)
    if key not in _NC_CACHE:
        nc = _build_nc()
        nc.compile()
        _NC_CACHE[key] = nc
    return _NC_CACHE[key]


def _make_band(scale):
    i = np.arange(DEP)
    band = (np.abs(i[:, None] - i[None, :]) <= R).astype(np.float32) * scale
    return np.ascontiguousarray(band.astype(np.float16))


def kernel(x, W=None, **_unused):
    global LAST_RESULT
    x = np.asarray(x, dtype=np.float32).reshape(B, DEP, HGT, WID)

    scale = 1.0
    if W is not None:
        scale = float(np.asarray(W, dtype=np.float32).ravel()[0])

    # Host-side shard: quantize, pad H and W by R with zeros, slice H
    # halves with halo.
    if INPUT_DT == "i8":
        s8 = float(np.max(np.abs(x))) / 127.0
        xq = np.clip(np.rint(x * (1.0 / s8)), -127, 127).astype(np.int8)
        band = _make_band(scale * s8)
    else:
        xq = x.astype(np.float16)
        band = _make_band(scale)

    nonce = np.zeros((1, _nonce_cols()), dtype=np.float32)
    in_maps = []
    for c in range(N_CORES):
        b, half = divmod(c, 2)
        xp = np.pad(xq[b], ((0, 0), (R, R), (R, R)))  # (128, 260, 260)
        h_start = half * H_HALF
        shard = np.ascontiguousarray(xp[:, h_start:h_start + H_IN, :])
        in_maps.append({"x": shard, "band": band, "nonce": nonce})

    nc = _get_nc()
    res = run_bass_kernel_spmd(
        nc, in_maps, core_ids=list(range(N_CORES)), trace=TRACE)
    LAST_RESULT = res

    out = np.empty((B, 1, DEP, HGT, WID), dtype=np.float32)
    for c in range(N_CORES):
        b, half = divmod(c, 2)
        h_start = half * H_HALF
        out[b, 0, :, h_start:h_start + H_HALF, :] = \
            res.results[c]["y"].astype(np.float32)
    return out


# revision 74
# speedup vs baseline: 1.0013x; 1.0013x over previous
"""Trainium2 Bass kernel for 5x5x5 all-ones Conv3d (box filter), stride 1, pad 2.

Input x: (4, 1, 128, 256, 256) fp32, W: (1,1,5,5,5) all-ones.
Output:  (4, 1, 128, 256, 256) fp32.

Strategy (8 NeuronCores): shard batch(4) x H-halves(2) -> 8 shards. The
all-ones conv is separable into three 5-tap box sums (W, H, D).

Pipeline (int8 input / fp16 output; fp32 where precision needs it):
  - Input quantized to int8 on host (one global scale s8 = absmax/127,
    folded into the band matrix), output fp16: 1 + 2 bytes per voxel of
    HBM traffic vs 4 + 4 fp32 -> per-core DMA ~12.9 MB ~36 us at the
    modeled 360 B/ns. All device arithmetic on exact small integers
    (scan state fp32, a/c sums < 2048 exact in fp16, PSUM fp32), so the
    only model error is the int8 quantization itself (~1.23e-2 norm).
    Set INPUT_DT="f16" for a ~5e-4-accurate variant (+12 us DMA).
  - W-axis 5-tap box in ONE DVE pass: tensor_tensor_scan with
    state = (x[t] + state) - x[t-5]  (data1 = the stream shifted by 5;
    the shift opens on zeroed lead columns in the chunk tile). The scan
    ISA op only exists on DVE, so DVE's ~36 us of scans is the floor
    the rest of the schedule is balanced around. Each chunk scans in
    two sub-scans (split at 7 rows, each with its own zero lead) so the
    c-pass and matmuls start before the whole chunk is scanned.
  - H-axis 5-tap = 3 PSUM-accumulating matmuls per 2-row group using
    pair sums: box5_h = c_{h-2} + c_h + a_{h+2},  c_h = a_h + a_{h+1}.
    Each matmul contracts D with the clipped banded all-ones matrix
    (s8-scaled) -> D-axis 5-tap fused into the same matmuls.
  - The c-pass is split column-wise: cols [0,96) on DVE (0.52 ns/elem
    in the 2x fp16 mode), cols [96,256) on Pool (gpsimd Add, ~1.98
    ns/elem) -> DVE ~45 us, Pool ~42 us, PE ~43 us, ACT ~35 us all
    land just under/around the pipeline's ~58 us span.
  - a (scan stream) and c live in persistent SBUF buffers -> chunk
    halos are plain backward reads, no copy traffic.
  - ScalarE (ACT) evicts 4-row PSUM tiles -> fp16 out tiles; in-DMA on
    the SP HWDGE ring, out-DMA (split at 8 rows) on the ACT ring.

Cost-model TimelineSim: 58.0 us/core (baseline fp32 kernel: 101.6 us).
Relative error 1.232e-2 (int8 quantization), under the 2e-2 gate.
"""

import numpy as np

import concourse.mybir as mybir
import concourse.tile as tile
from concourse import bacc
from concourse.bass_utils import run_bass_kernel_spmd

# Problem geometry (hardcoded; kernel.py must be self-contained).
B = 4
DEP = 128                  # depth (on partitions)
HGT = 256                  # height
WID = 256                  # width
KS = 5
R = 2                      # conv radius

N_CORES = 8
H_HALF = HGT // 2          # 128 output rows per core
H_IN = H_HALF + 2 * R      # 132 input rows per core
W_PAD = WID + 2 * R        # 260

# Per-chunk output rows (sum = 128). Tapered ends shrink pipeline
# fill/drain. The W-box scan is DVE-only (the scan ISA op doesn't
# exist on Pool); the c-pass is split column-wise between DVE
# (0.52 ns/elem, 2x fp16 mode) and Pool (~1.98 ns/elem gpsimd Add) so
# no engine exceeds the ~5.2 us/chunk steady pace.
CHUNKS = [4, 12, 16, 16, 16, 16, 16, 16, 8, 6, 2]
XT_MAX = 16                # max newly-loaded x rows per chunk
U_COLS = 0                 # columns [0, U): direct 5-tap matmuls, no c.
                           # Tested at 24: DVE busy drops as expected,
                           # but total worsens - PE's finish is
                           # onset+work bound and PE already trails DVE,
                           # so adding PE work extends the tail.
C_DVE_COLS = 94            # c columns [U_COLS, 94) on DVE; rest on Pool
SET_ROWS = 4               # rows per PSUM tile (2 banks; 2-row matmuls,
                           # one 4-row ACT evict)
PE_WARMUP = 0              # dummy matmuls pre-ramping the PE p-state
TAIL_FILL = True           # last two chunks' c all on DVE

INPUT_DT = "i8"            # "i8" (int8 input) | "f16"
REPEAT = 1                 # run the whole body N times (benchmarking only)
TRACE = False              # set True (from test.py) to profile
LAST_RESULT = None         # BassKernelResults of the last run (for test.py)

_NC_CACHE = {}

F16 = mybir.dt.float16


def _nonce_cols():
    key = (REPEAT, tuple(CHUNKS), C_DVE_COLS, INPUT_DT, SET_ROWS,
           PE_WARMUP, "v5")
    return 8 + hash(key) % 4093


def _build_nc():
    """Build the per-core Bass program (identical on all 8 cores)."""
    nc = bacc.Bacc("TRN2", target_bir_lowering=False, debug=False)

    in_dt = mybir.dt.int8 if INPUT_DT == "i8" else F16
    x_d = nc.dram_tensor("x", [DEP, H_IN, W_PAD], in_dt, kind="ExternalInput")
    band_d = nc.dram_tensor("band", [DEP, DEP], F16, kind="ExternalInput")
    # unused input whose shape encodes the config -> distinct HLO fingerprint
    # per kernel variant (defeats any shape-keyed executable caching)
    nc.dram_tensor("nonce", [1, _nonce_cols()], mybir.dt.float32,
                   kind="ExternalInput")
    y_d = nc.dram_tensor("y", [DEP, H_HALF, WID], F16, kind="ExternalOutput")

    ASTR_N = H_IN * W_PAD + 16   # slop: row-view slices of the last rows

    with tile.TileContext(nc) as tc:
        with (
            tc.tile_pool(name="const", bufs=1) as cpool,
            tc.tile_pool(name="xin", bufs=3) as xin_pool,
            tc.tile_pool(name="opool", bufs=4) as out_pool,
            tc.tile_pool(name="psum", bufs=4, space="PSUM") as ps_pool,
        ):
            band = cpool.tile([DEP, DEP], F16, name="band")

            # Persistent scan stream: a_h (W-box of x row hx=h+2) lives at
            # astr[hx*W_PAD + 4 + w], w in [0, WID). Persistent c buffer:
            # c_h = a_h + a_{h+1} at cbuf[:, h+2, :], h in [-2, 128).
            astr = cpool.tile([DEP, ASTR_N], F16, name="astr")
            cbuf = cpool.tile([DEP, H_HALF + 2, WID], F16, name="cbuf")

            # xin tile layout: [lead8][rows 0..7)[lead8][rows 7..). Both
            # 8-col zero leads let the two sub-scans run independently
            # (each data1 window opens on zeros). Chunk DMAs never write
            # the leads, so zeroing each pool buffer ONCE suffices.
            LEAD2 = 8 + 7 * W_PAD        # second lead offset (split at 7)
            XT_N = 16 + XT_MAX * W_PAD
            for _ in range(3):
                xt0 = xin_pool.tile([DEP, XT_N], in_dt, name="xt", tag="xt")
                nc.vector.memset(xt0[:, 0:8], 0.0)
                nc.vector.memset(xt0[:, LEAD2:LEAD2 + 8], 0.0)

            if PE_WARMUP:
                # PE p-state warmup: the cost model ramps the Tensor
                # engine (LOW -> MID -> full over ~3 us of continuous
                # work); zero x zero dummy matmuls through the fill
                # phase let the first real matmuls run at full rate.
                # Cost-neutral end-to-end (PE's finish is phase-bound),
                # so disabled by default.
                wt0 = cpool.tile([DEP, DEP], F16, name="wt0")
                rhs0 = cpool.tile([DEP, 512], F16, name="rhs0")
                nc.gpsimd.memset(wt0[:], 0.0)
                nc.gpsimd.memset(rhs0[:], 0.0)
                for _ in range(PE_WARMUP):
                    psw = ps_pool.tile([DEP, SET_ROWS, WID],
                                       mybir.dt.float32, name="ps",
                                       tag="ps")
                    nc.tensor.matmul(psw[:, 0:2, :], wt0[:], rhs0[:],
                                     start=True, stop=True)

            def arows(hx0, nr):
                """[DEP, nr, WID] view of a rows hx0..hx0+nr (x-row idx)."""
                b0 = hx0 * W_PAD + 4
                v = astr[:, b0:b0 + nr * W_PAD]
                return v.rearrange("q (r w) -> q r w", r=nr, w=W_PAD)[:, :, 0:WID]

            h0 = 0
            loaded = 0      # x rows loaded/scanned so far
            c_lo = -R       # next c row to compute
            for idx, oc in enumerate(CHUNKS * REPEAT):
                if idx % len(CHUNKS) == 0:
                    h0 = 0
                    loaded = 0
                    c_lo = -R

                # ---- load new x rows + W-box scan into astr (DVE) ----
                # Scans are independent at row boundaries: boxes never
                # span rows, and each sub-scan's data1 window opens on
                # its own zeroed lead columns, so the shifted-difference
                # recurrence starts clean (initial=0) in every sub-scan.
                hx_end = h0 + oc + 2 * R
                n_new = hx_end - loaded
                s0 = loaded
                xt = xin_pool.tile([DEP, XT_N], in_dt, name="xt", tag="xt")
                kA = min(n_new, 7)
                nc.sync.dma_start(out=xt[:, 8:8 + kA * W_PAD],
                                  in_=x_d[:, s0:s0 + kA, :])
                if kA < n_new:
                    nc.sync.dma_start(
                        out=xt[:, LEAD2 + 8:
                               LEAD2 + 8 + (n_new - kA) * W_PAD],
                        in_=x_d[:, s0 + kA:s0 + n_new, :])

                def sub_scan(r0, nr, base):
                    """Scan x rows [s0+r0, s0+r0+nr); `base` is the
                    tile offset of row r0's data (preceded by zeros)."""
                    nf = nr * W_PAD
                    p0 = (s0 + r0) * W_PAD
                    nc.vector.tensor_tensor_scan(
                        out=astr[:, p0:p0 + nf],
                        data0=xt[:, base:base + nf],
                        data1=xt[:, base - 5:base - 5 + nf],
                        initial=0.0,
                        op0=mybir.AluOpType.add,
                        op1=mybir.AluOpType.subtract,
                    )

                # all-DVE c for chunk 0 (Pool launch off the fill chain)
                # and optionally the last two chunks (Pool's slow final
                # c otherwise extends the drain cascade)
                ci = idx % len(CHUNKS)
                fill = ci == 0 or (TAIL_FILL and ci >= len(CHUNKS) - 2)

                def emit_c(upto):
                    """c rows [c_lo, upto): one DVE instruction for its
                    column share (fewer per-instruction inits; DVE never
                    gates the matmuls), <=8-row pieces for Pool's share
                    (Pool is slower, so its first piece should unblock
                    the chunk's first matmul sets early)."""
                    nonlocal c_lo
                    if c_lo >= upto:
                        return
                    nr_all = upto - c_lo
                    split = WID if fill else C_DVE_COLS
                    # cols [0, U_COLS) skip c entirely (5-tap zone)
                    nc.vector.tensor_add(
                        out=cbuf[:, c_lo + 2:c_lo + 2 + nr_all,
                                 U_COLS:split],
                        in0=arows(c_lo + 2, nr_all)[:, :, U_COLS:split],
                        in1=arows(c_lo + 3, nr_all)[:, :, U_COLS:split],
                    )
                    while c_lo < upto:
                        nr = min(8, upto - c_lo)
                        if split < WID:
                            nc.gpsimd.tensor_add(
                                out=cbuf[:, c_lo + 2:c_lo + 2 + nr,
                                         split:WID],
                                in0=arows(c_lo + 2, nr)[:, :, split:WID],
                                in1=arows(c_lo + 3, nr)[:, :, split:WID],
                            )
                        c_lo += nr

                c_hi = h0 + oc
                sub_scan(0, kA, 8)
                # c rows coverable with a rows hx < s0+kA: h+3 <= s0+kA
                emit_c(min(c_hi, s0 + kA - 3))
                if kA < n_new:
                    sub_scan(kA, n_new - kA, LEAD2 + 8)
                emit_c(c_hi)
                loaded = hx_end
                if idx == 0:
                    # band via the Pool SWDGE path: keeps the shared
                    # HWDGE queue free for chunk 0's x DMA (x is on the
                    # critical path; band isn't needed until the first
                    # matmul)
                    nc.gpsimd.dma_start(out=band[:], in_=band_d[:])

                # ---- D-sum + H-sum: 3 accumulating matmuls per 2-row
                # group; 4-row PSUM tiles evicted in one ACT copy ----
                out_t = out_pool.tile([DEP, 16, WID], F16,
                                      name="out_t", tag="out_t")
                r0 = h0
                while r0 < h0 + oc:
                    rows = min(SET_ROWS, h0 + oc - r0)
                    ps = ps_pool.tile([DEP, SET_ROWS, WID],
                                      mybir.dt.float32, name="ps", tag="ps")
                    for g0 in range(0, rows, 2):
                        gr = min(2, rows - g0)
                        r = r0 + g0
                        if U_COLS:
                            # cols [0, U_COLS): 5 direct a-taps (no c)
                            pu = ps[:, g0:g0 + gr, 0:U_COLS]
                            for j in range(KS):
                                nc.tensor.matmul(
                                    pu, band[:],
                                    arows(r + j, gr)[:, :, 0:U_COLS],
                                    start=(j == 0), stop=(j == KS - 1))
                        # cols [U_COLS, WID): c_{h-2}, c_h, a_{h+2}
                        pv = ps[:, g0:g0 + gr, U_COLS:WID]
                        nc.tensor.matmul(pv, band[:],
                                         cbuf[:, r:r + gr, U_COLS:WID],
                                         start=True, stop=False)
                        nc.tensor.matmul(pv, band[:],
                                         cbuf[:, r + 2:r + 2 + gr,
                                              U_COLS:WID],
                                         start=False, stop=False)
                        nc.tensor.matmul(pv, band[:],
                                         arows(r + 4, gr)[:, :,
                                                          U_COLS:WID],
                                         start=False, stop=True)
                    nc.scalar.copy(out=out_t[:, r0 - h0:r0 - h0 + rows, :],
                                   in_=ps[:, 0:rows, :])
                    r0 += rows
                    # out-DMA on the ACT HWDGE ring (separate FIFO from
                    # in-DMAs); split at 8 rows so the first half ships
                    # while the second half is still in the matmuls
                    if r0 - h0 == 8 and oc > 8:
                        nc.scalar.dma_start(out=y_d[:, h0:h0 + 8, :],
                                            in_=out_t[:, 0:8, :])
                done = 8 if oc > 8 else 0
                nc.scalar.dma_start(out=y_d[:, h0 + done:h0 + oc, :],
                                    in_=out_t[:, done:oc, :])
                h0 += oc

    return nc


def _get_nc():
    key = (REPEAT, tuple(CHUNKS), C_DVE_COLS, INPUT_DT, SET_ROWS,
           PE_WARMUP)
    if key not in _NC_CACHE:
        nc = _build_nc()
        nc.compile()
        _NC_CACHE[key] = nc
    return _NC_CACHE[key]


def _make_band(scale):
    i = np.arange(DEP)
    band = (np.abs(i[:, None] - i[None, :]) <= R).astype(np.float32) * scale
    return np.ascontiguousarray(band.astype(np.float16))


def kernel(x, W=None, **_unused):
    global LAST_RESULT
    x = np.asarray(x, dtype=np.float32).reshape(B, DEP, HGT, WID)

    scale = 1.0
    if W is not None:
        scale = float(np.asarray(W, dtype=np.float32).ravel()[0])

    # Host-side shard: quantize, pad H and W by R with zeros, slice H
    # halves with halo.
    if INPUT_DT == "i8":
        s8 = float(np.max(np.abs(x))) / 127.0
        xq = np.clip(np.rint(x * (1.0 / s8)), -127, 127).astype(np.int8)
        band = _make_band(scale * s8)
    else:
        xq = x.astype(np.float16)
        band = _make_band(scale)

    nonce = np.zeros((1, _nonce_cols()), dtype=np.float32)
    in_maps = []
    for c in range(N_CORES):
        b, half = divmod(c, 2)
        xp = np.pad(xq[b], ((0, 0), (R, R), (R, R)))  # (128, 260, 260)
        h_start = half * H_HALF
        shard = np.ascontiguousarray(xp[:, h_start:h_start + H_IN, :])
        in_maps.append({"x": shard, "band": band, "nonce": nonce})

    nc = _get_nc()
    res = run_bass_kernel_spmd(
        nc, in_maps, core_ids=list(range(N_CORES)), trace=TRACE)
    LAST_RESULT = res

    out = np.empty((B, 1, DEP, HGT, WID), dtype=np.float32)
    for c in range(N_CORES):
        b, half = divmod(c, 2)
        h_start = half * H_HALF
        out[b, 0, :, h_start:h_start + H_HALF, :] = \
            res.results[c]["y"].astype(np.float32)
    return out
